# revision 5
# baseline (speedup 1.0000x reference)
"""Multi-head self-attention kernel for Trainium2, batch-parallel over 8 NeuronCores.

Problem: B=8, S=1024, IN_DIM=D_MODEL=768, H=12, DK=64.
  q/k/v = Q @ W{q,k,v}.T + b   -> [b, H, s, dk]
  scores = exp(q k^T / 8) * key_mask ; attn = scores / sum
  out = attn @ v -> [b, s, 768]

Strategy (per core = one batch element):
  - host: QT = Q[b].T, WT = W.T (m-chunked), maskbias[p, t] = 0 / -60 from length
  - v in [s, d] layout with a ones-column appended per head (rowsum trick)
  - qT/kT in [d, s] layout, per head-pair (d-tile)
  - scoresT[sk, sq] via K=64 matmuls (two heads packed in PE rows 0-63 / 64-127)
  - exp fused with mask bias + 1/sqrt(dk) scale on ACT, writes bf16 probsT
  - ctx chains packed 7-per-PSUM-bank; batched normalize: one strided
    reciprocal per bank + stride-0-broadcast tensor_tensor multiplies
  - output DMA'd per head-pair column strip (overlapped with compute)
  - software pipeline: ctx of pair t-1 interleaved with scores of pair t
"""

import functools
import sys
import types

import numpy as np

B, S, IN_DIM, D_MODEL, H = 8, 1024, 768, 768, 12
DK = D_MODEL // H
NCORES = 8
NKT = IN_DIM // 128   # 6 contraction tiles
NDT = D_MODEL // 128  # 6 d-tiles (head pairs)
NST = S // 128        # 8 s-tiles
MASK_BIAS = -60.0


def _install_shims():
    """antenv.axon_hooks shim (for NTFF tracing) + Tile drain-wait splitting
    (this walrus build accepts only one sync-wait command per Drain/CTRL)."""
    if 'antenv.axon_hooks' not in sys.modules:
        mod = types.ModuleType('antenv.axon_hooks')
        mod._hook = None
        mod.set_axon_ntff_profile_hook = lambda h: setattr(mod, '_hook', h)
        mod.get_axon_ntff_profile_hook = lambda: mod._hook
        sys.modules['antenv.axon_hooks'] = mod
        try:
            import antenv
            antenv.axon_hooks = mod
            from trn_agent_boot.trn_boot import _ntff_profile_via_ctypes
            mod.set_axon_ntff_profile_hook(
                _ntff_profile_via_ctypes('/opt/axon/libaxon_pjrt.so'))
        except Exception:
            pass

    import concourse.tile as tile
    if getattr(tile.TileContext, '_drain_patched', False):
        return
    from concourse.vector_clock import ScopedClock, VectorClock

    def _patched_drain_and_barrier(self, tick_clock, wait_clock):
        nc = self.nc
        gvec = tick_clock.global_clock
        n = len(gvec)
        for i in range(n):
            t = gvec[i]
            if t <= 0:
                continue
            v = [0] * n
            v[i] = t
            nop = nc.sync.nop(nofuse=True, hint="drain_wait_split")
            wait_clock.add_sem_waits(nop.ins, ScopedClock({None: VectorClock(v)}))
        # The per-proc NOPs above carry every wait (SP queue is in-order),
        # so the drain itself needs none.
        nc.sync.drain()
        nc.all_engine_barrier()
        assert self.sems is not None
        popped = nc._tile_sem_poison_stack.pop()
        assert popped is self._sem_poison
        nc.clear_and_free_semaphores(list(self.sems.allocated().values()))
        nc.all_engine_barrier()

    tile.TileContext._drain_and_barrier = _patched_drain_and_barrier

    # This walrus build accepts at most ONE sync-wait command per engine
    # instruction: split extra waits onto non-fusable NOPs emitted just
    # before the instruction on the same engine queue.
    import bass_rust
    import concourse.mybir as mybir
    _orig_lower = tile.TileContext._lower_ordered_insts

    def _split_waits_then_lower(self, ordered):
        nc = self.nc
        for bbname, insts in ordered.items():
            need = any(
                i.sync_info is not None and i.sync_info.on_wait
                and len(i.sync_info.on_wait) > 1
                for i in insts)
            if not need:
                continue
            out = []
            for inst in insts:
                si = inst.sync_info
                if si is not None and si.on_wait and len(si.on_wait) > 1:
                    waits = list(si.on_wait)
                    for w in waits[:-1]:
                        nop = mybir.InstNoOp(
                            name=nc.get_next_instruction_name(), ins=[], outs=[])
                        nop.engine = inst.engine
                        nop.bass_nofuse = True
                        nop.sync_info = bass_rust.SyncInfo(
                            on_wait=[w], on_update=[])
                        out.append(nop)
                    inst.sync_info = bass_rust.SyncInfo(
                        on_wait=[waits[-1]],
                        on_update=list(si.on_update or []))
                out.append(inst)
            insts[:] = out
        return _orig_lower(self, ordered)

    tile.TileContext._lower_ordered_insts = _split_waits_then_lower
    tile.TileContext._drain_patched = True


@functools.lru_cache(maxsize=None)
def _build_program(n_sk: int, use_bias: bool):
    import concourse.bass as bass
    import concourse.tile as tile
    import concourse.mybir as mybir
    from contextlib import ExitStack

    f32 = mybir.dt.float32
    bf16 = mybir.dt.bfloat16
    EXP = mybir.ActivationFunctionType.Exp
    MULT = mybir.AluOpType.mult

    nc = bass.Bass("TRN2", enable_partition_id=False)
    qt_d = nc.dram_tensor("qt", (IN_DIM, S), bf16, kind="ExternalInput")
    wqm_d = nc.dram_tensor("wqm", (NDT, IN_DIM, 128), bf16, kind="ExternalInput")
    wkm_d = nc.dram_tensor("wkm", (NDT, IN_DIM, 128), bf16, kind="ExternalInput")
    wvt_d = nc.dram_tensor("wvt", (IN_DIM, D_MODEL), bf16, kind="ExternalInput")
    mb_d = nc.dram_tensor("mb", (128, NST), f32, kind="ExternalInput")
    if use_bias:
        bq_d = nc.dram_tensor("bq", (1, D_MODEL), bf16, kind="ExternalInput")
        bk_d = nc.dram_tensor("bk", (1, D_MODEL), bf16, kind="ExternalInput")
        bv_d = nc.dram_tensor("bv", (1, D_MODEL), bf16, kind="ExternalInput")
    out_d = nc.dram_tensor("out", (S, D_MODEL), f32, kind="ExternalOutput")

    with tile.TileContext(nc) as tc, ExitStack() as ctx:
        const = ctx.enter_context(tc.tile_pool(name="const", bufs=1))
        big = ctx.enter_context(tc.tile_pool(name="big", bufs=1))
        wpool = ctx.enter_context(tc.tile_pool(name="w", bufs=3))
        qkpool = ctx.enter_context(tc.tile_pool(name="qk", bufs=3))
        prpool = ctx.enter_context(tc.tile_pool(name="pr", bufs=1))
        smpool = ctx.enter_context(tc.tile_pool(name="sm", bufs=6))
        pj = ctx.enter_context(tc.tile_pool(name="pj", bufs=2, space="PSUM"))
        sc = ctx.enter_context(tc.tile_pool(name="sc", bufs=2, space="PSUM"))
        cx = ctx.enter_context(tc.tile_pool(name="cx", bufs=1, space="PSUM"))

        # ---- input DMAs; mb first (tiny, needed by first exp), weights for
        # pair 0, then qt (fine-grained per-k-tile deps let the first qkproj
        # matmuls start as soon as wq0 + qt0 land), wvt last (only needed
        # once vproj starts mid-pair-0).
        mb_sb = const.tile([128, NST], f32)
        nc.gpsimd.dma_start(out=mb_sb, in_=mb_d[:, :])

        wqk_pending = {}

        def prefetch_wqk(t):
            wq_sb = wpool.tile([128, NKT, 128], bf16, tag="wq", name=f"wq{t}")
            nc.sync.dma_start(
                out=wq_sb, in_=wqm_d[t].rearrange("(k p) m -> p k m", p=128))
            wk_sb = wpool.tile([128, NKT, 128], bf16, tag="wk", name=f"wk{t}")
            nc.sync.dma_start(
                out=wk_sb, in_=wkm_d[t].rearrange("(k p) m -> p k m", p=128))
            wqk_pending[t] = (wq_sb, wk_sb)

        prefetch_wqk(0)
        qt_sb = []
        qt_engs = [nc.gpsimd, nc.sync, nc.scalar, nc.gpsimd, nc.sync, nc.scalar]
        for k in range(NKT):
            qk_t = big.tile([128, S], bf16, name=f"qtsb{k}")
            qt_engs[k].dma_start(out=qk_t, in_=qt_d[k * 128:(k + 1) * 128, :])
            qt_sb.append(qk_t)
        wvt_sb = []
        wvt_engs = [nc.gpsimd, nc.sync, nc.scalar, nc.gpsimd, nc.sync, nc.scalar]
        for k in range(NKT):
            wv_t = big.tile([128, D_MODEL], bf16, name=f"wvtsb{k}")
            wvt_engs[k].dma_start(out=wv_t, in_=wvt_d[k * 128:(k + 1) * 128, :])
            wvt_sb.append(wv_t)
        v_sb = big.tile([128, NST, H * (DK + 1)], bf16)
        out_sb = big.tile([128, NST, D_MODEL], f32)
        if use_bias:
            ones_sb = const.tile([1, 512], bf16)
            nc.vector.memset(ones_sb, 1.0)
            bq_sb = const.tile([1, D_MODEL], bf16)
            nc.sync.dma_start(out=bq_sb, in_=bq_d[:, :])
            bk_sb = const.tile([1, D_MODEL], bf16)
            nc.sync.dma_start(out=bk_sb, in_=bk_d[:, :])
            bv_sb = const.tile([1, D_MODEL], bf16)
            nc.sync.dma_start(out=bv_sb, in_=bv_d[:, :])

        # ---- v projection: [s, d] layout, heads strided by 65 with ones col
        def emit_vproj(srow):
            for nch in range(2):  # 384 cols = 6 heads each
                ps = pj.tile([128, 384], f32, tag="px", name=f"psv{srow}_{nch}")
                for k in range(NKT):
                    nc.tensor.matmul(
                        ps,
                        lhsT=qt_sb[k][:, srow * 128:(srow + 1) * 128],
                        rhs=wvt_sb[k][:, nch * 384:(nch + 1) * 384],
                        start=(k == 0), stop=(k == NKT - 1 and not use_bias))
                if use_bias:
                    nc.tensor.matmul(
                        ps, lhsT=ones_sb[0:1, 0:128],
                        rhs=bv_sb[0:1, nch * 384:(nch + 1) * 384],
                        start=False, stop=True)
                dst = v_sb[:, srow, nch * 390:(nch + 1) * 390]
                dst3 = dst.rearrange("p (h x) -> p h x", x=DK + 1)[:, :, 0:DK]
                src3 = ps.rearrange("p (h x) -> p h x", x=DK)
                nc.vector.tensor_copy(out=dst3, in_=src3)
            ones_dst = v_sb[:, srow, :].rearrange(
                "p (h x) -> p h x", x=DK + 1)[:, :, DK:DK + 1]
            nc.vector.memset(ones_dst, 1.0)

        # ---- per head-pair machinery
        def emit_qkproj(t):
            wq_sb, wk_sb = wqk_pending.pop(t)
            qT = qkpool.tile([128, S], bf16, tag="qT", name=f"qT{t}")
            kT = qkpool.tile([128, S], bf16, tag="kT", name=f"kT{t}")
            for w_sb, b_sb, dstT, nm in ((wq_sb, "bq", qT, "q"), (wk_sb, "bk", kT, "k")):
                for nch in range(2):
                    ps = pj.tile([128, 512], f32, tag="px", name=f"ps{nm}{t}_{nch}")
                    for k in range(NKT):
                        nc.tensor.matmul(
                            ps,
                            lhsT=w_sb[:, k, :],
                            rhs=qt_sb[k][:, nch * 512:(nch + 1) * 512],
                            start=(k == 0), stop=(k == NKT - 1 and not use_bias))
                    if use_bias:
                        bias_sb = bq_sb if b_sb == "bq" else bk_sb
                        nc.tensor.matmul(
                            ps,
                            lhsT=bias_sb[0:1, t * 128:(t + 1) * 128],
                            rhs=ones_sb[0:1, 0:512],
                            start=False, stop=True)
                    nc.vector.tensor_copy(
                        out=dstT[:, nch * 512:(nch + 1) * 512], in_=ps)
            return qT, kT

        probs = {}

        def emit_scores_sk(t, sk, qT, kT):
            pss = []
            for hl in range(2):
                pss.append(sc.tile([128, S], f32, tag="sc",
                                   name=f"sc{t}_{sk}_{hl}"))
            for hl in range(2):
                lo, hi = hl * 64, (hl + 1) * 64
                for nch in range(2):
                    nc.tensor.matmul(
                        pss[hl][:, nch * 512:(nch + 1) * 512],
                        lhsT=kT[lo:hi, sk * 128:(sk + 1) * 128],
                        rhs=qT[lo:hi, nch * 512:(nch + 1) * 512],
                        start=True, stop=True)
            for hl in range(2):
                pb = prpool.tile([128, S], bf16, tag=f"pb{t % 2}_{hl}_{sk}",
                                 name=f"pb{t}_{hl}_{sk}")
                nc.scalar.activation(
                    out=pb, in_=pss[hl], func=EXP,
                    bias=mb_sb[:, sk:sk + 1], scale=1.0 / np.sqrt(DK))
                probs[(t % 2, hl, sk)] = pb

        # ctx batches: per pair, 16 chains (hl, sq) of 65 psum cols each,
        # packed 7 + 7 + 2 into three bank-tiles (tags cxA, cxB, cxA).
        CHAINS = [(hl, sq) for hl in range(2) for sq in range(NST)]
        BATCHES = [(0, 7, "cxA"), (7, 14, "cxB"), (14, 16, "cxA")]

        def emit_ctx_batch(t, bi):
            c0, c1, tag = BATCHES[bi]
            nch = c1 - c0
            pc = cx.tile([128, nch * (DK + 1)], f32, tag=tag,
                         name=f"cx{t}_{bi}")
            for ci in range(nch):
                hl, sq = CHAINS[c0 + ci]
                head = 2 * t + hl
                for sk in range(n_sk):
                    nc.tensor.matmul(
                        pc[:, ci * (DK + 1):ci * (DK + 1) + DK + 1],
                        lhsT=probs[(t % 2, hl, sk)][:, sq * 128:(sq + 1) * 128],
                        rhs=v_sb[:, sk, head * (DK + 1):(head + 1) * (DK + 1)],
                        start=(sk == 0), stop=(sk == n_sk - 1))
            return pc

        def emit_ctx_normalize(t, bi, pc):
            c0, c1, tag = BATCHES[bi]
            nch = c1 - c0
            pc3 = pc.rearrange("p (c x) -> p c x", x=DK + 1)
            rec = smpool.tile([128, nch], f32, name=f"rec{t}_{bi}")
            nc.vector.reciprocal(rec, pc3[:, :, DK])
            # runs of consecutive chains with the same head
            ci = 0
            while ci < nch:
                hl0 = CHAINS[c0 + ci][0]
                cj = ci
                while cj < nch and CHAINS[c0 + cj][0] == hl0:
                    cj += 1
                cnt = cj - ci
                head = 2 * t + hl0
                sq0 = CHAINS[c0 + ci][1]
                out_ap = out_sb[:, sq0:sq0 + cnt, head * DK:(head + 1) * DK]
                in0 = pc3[:, ci:cj, 0:DK]
                in1 = rec[:, ci:cj].unsqueeze(-1).to_broadcast([128, cnt, DK])
                nc.vector.tensor_tensor(out=out_ap, in0=in0, in1=in1, op=MULT)
                ci = cj

        def emit_out_strip(t):
            # pair t's 128 output columns for all 1024 rows, 2 DMAs
            for half in range(2):
                rows = out_d[half * 512:(half + 1) * 512,
                             t * 128:(t + 1) * 128]
                nc.gpsimd.dma_start(
                    out=rows.rearrange("(s p) c -> p s c", p=128),
                    in_=out_sb[:, half * 4:(half + 1) * 4,
                               t * 128:(t + 1) * 128])

        # ---- main pipeline
        cur = emit_qkproj(0)
        prefetch_wqk(1)
        vi = 0
        for sk in range(n_sk):
            emit_scores_sk(0, sk, *cur)
            while vi < NST * (sk + 1) // n_sk:
                emit_vproj(vi)
                vi += 1
        while vi < NST:
            emit_vproj(vi)
            vi += 1

        for t in range(1, NDT):
            cur = emit_qkproj(t)
            if t + 1 < NDT:
                prefetch_wqk(t + 1)
            pcs = {}
            done = [False] * 4  # b0 mm, b0+b1 norm..., track emitted stages
            for sk in range(n_sk):
                emit_scores_sk(t, sk, *cur)
                if sk >= min(0, n_sk - 1) and not done[0]:
                    pcs[0] = emit_ctx_batch(t - 1, 0)
                    done[0] = True
                if sk >= min(2, n_sk - 1) and not done[1]:
                    emit_ctx_normalize(t - 1, 0, pcs[0])
                    pcs[1] = emit_ctx_batch(t - 1, 1)
                    done[1] = True
                if sk >= min(4, n_sk - 1) and not done[2]:
                    emit_ctx_normalize(t - 1, 1, pcs[1])
                    pcs[2] = emit_ctx_batch(t - 1, 2)
                    done[2] = True
                if sk >= min(6, n_sk - 1) and not done[3]:
                    emit_ctx_normalize(t - 1, 2, pcs[2])
                    emit_out_strip(t - 1)
                    done[3] = True

        # epilogue: ctx of the last pair
        t = NDT - 1
        pc0 = emit_ctx_batch(t, 0)
        emit_ctx_normalize(t, 0, pc0)
        pc1 = emit_ctx_batch(t, 1)
        emit_ctx_normalize(t, 1, pc1)
        pc2 = emit_ctx_batch(t, 2)
        emit_ctx_normalize(t, 2, pc2)
        emit_out_strip(t)

    return nc


TRACE = False
LAST_EXEC_NS = None
LAST_RES = None


def kernel(Q, length, Wq, bq, Wk, bk, Wv, bv):
    global LAST_EXEC_NS, LAST_RES
    _install_shims()
    from concourse.bass_utils import run_bass_kernel_spmd

    Q = np.asarray(Q, np.float32)
    length = np.asarray(length, np.int32)
    Wq, Wk, Wv = (np.asarray(w, np.float32) for w in (Wq, Wk, Wv))
    bq, bk, bv = (np.asarray(b, np.float32) for b in (bq, bk, bv))

    use_bias = bool(np.any(bq) or np.any(bk) or np.any(bv))
    maxlen = int(length.max()) if length.size else S
    n_sk = max(1, min(NST, -(-max(1, maxlen) // 128)))

    import ml_dtypes
    bfl = ml_dtypes.bfloat16
    qt_all = np.ascontiguousarray(Q.transpose(0, 2, 1)).astype(bfl)   # [B, 768, 1024]
    wqm = np.ascontiguousarray(Wq.T.reshape(IN_DIM, NDT, 128).transpose(1, 0, 2)).astype(bfl)
    wkm = np.ascontiguousarray(Wk.T.reshape(IN_DIM, NDT, 128).transpose(1, 0, 2)).astype(bfl)
    wvt = np.ascontiguousarray(Wv.T).astype(bfl)                      # [768, 768]
    j = np.arange(S)
    mb = np.where(j[None, :] < length[:, None], 0.0, MASK_BIAS).astype(np.float32)
    mb = np.ascontiguousarray(mb.reshape(B, NST, 128).transpose(0, 2, 1))  # [B,128,8]

    nc = _build_program(n_sk, use_bias)
    in_maps = []
    for b in range(B):
        m = {"qt": qt_all[b], "wqm": wqm, "wkm": wkm, "wvt": wvt, "mb": mb[b]}
        if use_bias:
            m["bq"] = bq.reshape(1, -1).astype(np.float32).astype(bfl)
            m["bk"] = bk.reshape(1, -1).astype(np.float32).astype(bfl)
            m["bv"] = bv.reshape(1, -1).astype(np.float32).astype(bfl)
        in_maps.append(m)

    res = run_bass_kernel_spmd(
        nc, in_maps, core_ids=list(range(NCORES)), trace=TRACE)
    LAST_EXEC_NS = res.exec_time_ns
    LAST_RES = res
    out = np.stack([res.results[b]["out"] for b in range(B)])
    out = np.ascontiguousarray(out.astype(np.float32))
    # reference: attn = p / (sum + 1e-8); for length==0 every key is masked
    # and the reference output is ~0, while our reciprocal normalization
    # averages the tiny masked probs. Zero those rows host-side.
    for b in range(B):
        if int(length[b]) == 0:
            out[b] = 0.0
    return out
